# revision 11
# baseline (speedup 1.0000x reference)
"""BWGNN_Hetero Trainium2 kernel v2: 8-core SPMD, node-sharded graph parallel.

Structure (per core):
  Phase A: fused fp16 LSTMs (voc+sms packed in one 128-row tile) -> linear
  head -> x_in[0..2] (feat-major fp16) to DRAM + out_parts[3] partials.

  Phase B: 6 (hop, relation) stages interleaved as (0,r0)(0,r1)(0,r2)
  (1,r0)(1,r1) E(r0) (1,r2) E(r1) E(r2) so the gpsimd dma_gather
  descriptor generation (the hard bottleneck, ~8.4ns/edge) streams
  continuously while PE/DVE/ACT work hides underneath. AllGathers for a
  stage are issued as soon as its messages exist (end of the producing
  stage), hiding collective latency.

  Aggregation: fp16 message AllGather -> dma_gather by src (1024-idx
  calls) -> one-hot indicator matmuls (f32 PSUM) as the dst segment sum.
  Indicators are generated on device (DVE is_equal vs an iota row,
  broadcast APs) instead of loaded from HBM.

  Epilogue per relation: single merged pass: transposes -> 5-filter
  scores -> softmax via one [5x4] matmul (den+3 gammas) -> combine ->
  lin5 -> lin6 partials.

Host sums partials, adds b_lin6, unshards.
"""
import sys
sys.path.insert(0, '/opt/trn_rl_repo')
sys.path.insert(0, '/root/problem')

import numpy as np

import concourse.bacc as bacc
import concourse.bass as bass
import concourse.mybir as mybir
import concourse.tile as tile
from concourse.bass_utils import run_bass_kernel_spmd

F32 = mybir.dt.float32
F16 = mybir.dt.float16
I16 = mybir.dt.int16
A = mybir.AluOpType
AF = mybir.ActivationFunctionType

NCORES = 8
N, E, R, T = 50000, 800000, 3, 16
IV, IS, IP, H, C = 64, 64, 32, 128, 2
NL = N // NCORES            # 6250
NT = 49                     # dst tiles per core
NLP = NT * 128              # 6272 padded local nodes
NGP = NLP * NCORES          # 50176 padded global rows in AllGather output
LO_LIM = 32768              # int16 gather index limit
GS = 4                      # dst tiles per gather group
GW = 8                      # windows per dma_gather call (1024 idx ring cap)
CHUNKS = [(i * 512, 512) for i in range(12)] + [(6144, 128)]

CTRUE = [[0.8, -0.5, 0.0],
         [3.0, -3.0, 0.75],
         [0.0, 3.0, -1.5],
         [0.0, 0.0, 0.75],
         [-0.2, 0.5, 0.0]]


def _wrap_idx(idx):
    """[n] int16 -> [128, ceil(n/16)] wrapped (i -> [i%16, i//16]) + replicated x8."""
    n = len(idx)
    L = max(1, (n + 15) // 16)
    a = np.zeros((16, L), np.int16)
    for p in range(16):
        vals = idx[p::16]
        a[p, :len(vals)] = vals
    return np.tile(a, (8, 1))


class WPack:
    def __init__(self):
        self.cols = []
        self.off = 0
        self.slots = {}

    def add(self, name, mat, row0=0):
        mat = np.asarray(mat, np.float32)
        k, m = mat.shape
        assert row0 + k <= 128
        buf = np.zeros((128, m), np.float16)
        buf[row0:row0 + k] = mat.astype(np.float16)
        self.cols.append(buf)
        self.slots[name] = (row0, k, self.off, m)
        self.off += m

    def image(self):
        return np.concatenate(self.cols, axis=1)


def _prep(inp):
    g = {k: np.asarray(v) for k, v in inp.items()}
    wp = WPack()

    groff = {0: 0, 1: 64, 2: 192, 3: 128}   # our gate order [i, f, o, gg] -> torch rows
    lv = np.zeros((128, 256), np.float32)
    ls = np.zeros((128, 256), np.float32)
    for gi in range(4):
        ro = groff[gi]
        lv[0:64, gi * 64:(gi + 1) * 64] = g['Whh_v'][ro:ro + 64, :].T
        lv[64:128, gi * 64:(gi + 1) * 64] = g['Wih_v'][ro:ro + 64, :].T
        ls[0:64, gi * 64:(gi + 1) * 64] = g['Wih_s'][ro:ro + 64, :].T
        ls[64:128, gi * 64:(gi + 1) * 64] = g['Whh_s'][ro:ro + 64, :].T
    wp.add('lstm_v', lv)
    wp.add('lstm_s', ls)
    wp.add('lin', g['W_lin'].T)                       # rows 0:64 (rhs = h_v at base 0)
    wp.add('lin1', g['W_lin1'].T, row0=64)            # rows 64:128 (rhs = h_s at base 64)
    wp.add('pers', g['W_pers'].T)
    wp.add('lin2a', g['W_lin2'][:, 0:128].T)
    wp.add('lin2b', g['W_lin2'][:, 128:256].T)
    wp.add('lin3a', g['W_lin3'][:, 0:128].T)
    wp.add('lin3b', g['W_lin3'][:, 128:256].T)
    wp.add('lin4a', g['W_lin4'][:, 0:128].T)
    wp.add('lin4bd', (g['W_lin4'][:, 128:256] + g['W_lin4'][:, 384:512]).T)
    wp.add('lin4c', g['W_lin4'][:, 256:384].T)
    for r in range(R):
        for o in range(5):
            for j in range(3):
                if CTRUE[o][j] != 0.0:
                    wp.add(f'wf1_{r}_{o}_{j}', (CTRUE[o][j] * g['Wf1'][r]).T)
        wp.add(f'wf2_{r}', g['Wf2'][r][:, None])
        wp.add(f'lin5_{r}', g['W_lin5'][r].T)
    for k in range(6):
        wp.add(f'lin6_{k}', g['W_lin6'][:, k * 128:(k + 1) * 128].T)
    wp.add('ident', np.eye(128, dtype=np.float32))
    # softmax combine rows: per filter o a [1,128] row; den coeff at col 0,
    # gamma_j coeffs at cols 32/64/96 (partition-aligned PSUM rows)
    for o in range(5):
        row = np.zeros((1, 128), np.float32)
        row[0, 0] = 1.0
        for j in range(3):
            row[0, 32 * (j + 1)] = CTRUE[o][j]
        wp.add(f'c5_{o}', row)
    wp.add('iota', np.tile(np.arange(128, dtype=np.float32), (128, 1)))
    wimg = wp.image()

    bcols, blist = {}, []

    def addb(name, vec):
        bcols[name] = len(blist)
        v = np.zeros((128, 1), np.float32)
        v[:len(vec), 0] = np.asarray(vec, np.float32).ravel()
        blist.append(v)

    bv = g['bih_v'] + g['bhh_v']
    bs = g['bih_s'] + g['bhh_s']
    for gi in range(4):
        ro = groff[gi]
        addb(f'bg{gi}', np.concatenate([bv[ro:ro + 64], bs[ro:ro + 64]]))
    addb('b_lin', g['b_lin'])
    addb('b_lin1', g['b_lin1'])
    addb('b_pers', g['b_pers'])
    addb('b2', g['b_lin2'])
    addb('b3', g['b_lin3'])
    addb('b4', g['b_lin4'])
    for r in range(R):
        addb(f'bf1_{r}', g['bf1'][r])
        addb(f'b5_{r}', g['b_lin5'][r])
    bimg = np.concatenate(blist, axis=1)

    src = np.asarray(g['src'], np.int64)
    dst = np.asarray(g['dst'], np.int64)
    gsrc_all = (src // NL) * NLP + (src % NL)

    percore = [dict() for _ in range(NCORES)]
    relmeta = []
    for r in range(R):
        deg = np.bincount(dst[r], minlength=N).astype(np.float32)
        dinv = np.clip(deg, 1.0, None) ** -0.5

        # bucket edges: (core, tile, class)
        per = []
        for c in range(NCORES):
            m = (dst[r] // NL) == c
            sc = gsrc_all[r][m]
            dl = dst[r][m] - c * NL
            tl, col = dl // 128, dl % 128
            tiles = []
            for t in range(NT):
                mt = tl == t
                st_, ct_ = sc[mt], col[mt]
                lo = st_ < LO_LIM
                tiles.append((st_[lo], ct_[lo], st_[~lo] - LO_LIM, ct_[~lo]))
            per.append(tiles)
        # common (max-over-cores) window counts
        lo_w = [max(1, max((len(per[c][t][0]) + 127) // 128 for c in range(NCORES)))
                for t in range(NT)]
        hi_w = [max(1, max((len(per[c][t][2]) + 127) // 128 for c in range(NCORES)))
                for t in range(NT)]
        relmeta.append({'lo_w': lo_w, 'hi_w': hi_w})
        for c in range(NCORES):
            li_s, lc_s, hi_s, hc_s = [], [], [], []
            for t in range(NT):
                li, lc, hi, hc = per[c][t]
                lp = np.zeros(lo_w[t] * 128, np.int64); lp[:len(li)] = li
                lcp = np.full(lo_w[t] * 128, -1, np.int64); lcp[:len(lc)] = lc
                hp = np.zeros(hi_w[t] * 128, np.int64); hp[:len(hi)] = hi
                hcp = np.full(hi_w[t] * 128, -1, np.int64); hcp[:len(hc)] = hc
                li_s.append(lp); lc_s.append(lcp); hi_s.append(hp); hc_s.append(hcp)
            li_s = np.concatenate(li_s); lc_s = np.concatenate(lc_s)
            hi_s = np.concatenate(hi_s); hc_s = np.concatenate(hc_s)

            def mkcol(colarr):
                W = len(colarr) // 128
                return colarr.reshape(W, 128).T.astype(np.float16)
            pc = percore[c]
            pc[f'gidx_lo_{r}'] = _wrap_idx(li_s.astype(np.int16))
            pc[f'gidx_hi_{r}'] = _wrap_idx(hi_s.astype(np.int16))
            pc[f'col_lo_{r}'] = mkcol(lc_s)
            pc[f'col_hi_{r}'] = mkcol(hc_s)
            dp = pc.setdefault('_dinv', np.zeros((128, 2 * R * NT), np.float32))
            dvl = np.ones(NLP, np.float32)
            dvl[:NL] = dinv[c * NL:(c + 1) * NL]
            dp[:, r * NT:(r + 1) * NT] = dvl.reshape(NT, 128).T
            dp[:, R * NT + r * NT:R * NT + (r + 1) * NT] = -dvl.reshape(NT, 128).T

    voc = np.asarray(g['voc_features'], np.float32)
    sms = np.asarray(g['sms_features'], np.float32)
    pers = np.asarray(g['personal_feature'], np.float32)
    cores = []
    for c in range(NCORES):
        pc = percore[c]
        sl = slice(c * NL, (c + 1) * NL)
        vt = np.zeros((T, IV, NLP), np.float16)
        st_ = np.zeros((T, IS, NLP), np.float16)
        vt[:, :, :NL] = voc[sl].transpose(1, 2, 0).astype(np.float16)
        st_[:, :, :NL] = sms[sl].transpose(1, 2, 0).astype(np.float16)
        pt = np.zeros((IP, NLP), np.float16)
        pt[:, :NL] = pers[sl].T.astype(np.float16)
        pc['voc'] = vt
        pc['sms'] = st_
        pc['pers'] = pt
        pc['wpack'] = wimg
        pc['bpack'] = bimg
        pc['dpack'] = pc.pop('_dinv')
        cores.append(pc)
    meta = {
        'wp': wp.slots, 'bcols': bcols, 'rel': relmeta,
        'shapes': {k: v.shape for k, v in cores[0].items()},
        'dtypes': {k: v.dtype for k, v in cores[0].items()},
    }
    return meta, cores


def _build(nc, meta):
    sh, dt = meta['shapes'], meta['dtypes']
    WP, BC = meta['wp'], meta['bcols']
    inputs = {k: nc.dram_tensor(k, list(sh[k]), mybir.dt.from_np(np.dtype(dt[k])),
                                kind="ExternalInput") for k in sh}
    out_parts = nc.dram_tensor("out_parts", [4, 2, NLP], F32, kind="ExternalOutput")

    xin = [nc.dram_tensor(f"xin{r}", [128, NLP], F16, kind="Internal")
           for r in range(R)]
    f1d = [nc.dram_tensor(f"f1d{r}", [128, NLP], F16, kind="Internal")
           for r in range(R)]
    f2d = [nc.dram_tensor(f"f2d{r}", [128, NLP], F16, kind="Internal")
           for r in range(R)]
    mld = {(r, h): nc.dram_tensor(f"ml{r}{h}", [NLP, H], F16, kind="Internal")
           for r in range(R) for h in range(2)}
    mfd = {(r, h): nc.dram_tensor(f"mf{r}{h}", [NGP, H], F16, kind="Internal",
                                  addr_space="Shared")
           for r in range(R) for h in range(2)}

    def wsl(wt, name):
        r0, k, off, m = WP[name]
        return wt[r0:r0 + k, off:off + m]

    with tile.TileContext(nc) as tc:
        with (
            tc.tile_pool(name="const", bufs=1) as cpool,
        ):
            wt = cpool.tile([128, sh['wpack'][1]], F16)
            nc.sync.dma_start(wt[:], inputs['wpack'][:])
            bt = cpool.tile([128, sh['bpack'][1]], F32)
            nc.sync.dma_start(bt[:], inputs['bpack'][:])
            dpt = cpool.tile([128, 2 * R * NT], F32)
            nc.sync.dma_start(dpt[:], inputs['dpack'][:])
            onesf16 = cpool.tile([1, 128], F16)
            nc.vector.memset(onesf16[:], 1.0)

            def bias(name):
                return bt[:, BC[name]:BC[name] + 1]

            def dv(r, t):
                return dpt[:, r * NT + t:r * NT + t + 1]

            def ndv(r, t):
                return dpt[:, R * NT + r * NT + t:R * NT + r * NT + t + 1]

            # =============== Phase A ===============
            with (tc.tile_pool(name="stA", bufs=1) as stA,
                  tc.tile_pool(name="wpA", bufs=2) as wpA):
                XHv = stA.tile([128, NLP], F16)
                XHs = stA.tile([128, NLP], F16)
                Cst = stA.tile([128, NLP], F16)
                nc.vector.memset(XHv[0:64, :], 0.0)
                nc.vector.memset(XHs[64:128, :], 0.0)
                nc.vector.memset(Cst[:], 0.0)
                with tc.tile_pool(name="psA", bufs=2, space="PSUM") as psA:
                    for t in range(T):
                        nc.sync.dma_start(XHv[64:128, :], inputs['voc'][t])
                        nc.sync.dma_start(XHs[0:64, :], inputs['sms'][t])
                        for (c0, cw) in CHUNKS:
                            P = [psA.tile([128, 512], F32, tag=f"g{gi}", name=f"Pg{gi}") for gi in range(4)]
                            for gi in range(4):
                                nc.tensor.matmul(P[gi][0:64, :cw],
                                                 lhsT=wsl(wt, 'lstm_v')[:, gi * 64:(gi + 1) * 64],
                                                 rhs=XHv[:, c0:c0 + cw], start=True, stop=True)
                                nc.tensor.matmul(P[gi][64:128, :cw],
                                                 lhsT=wsl(wt, 'lstm_s')[:, gi * 64:(gi + 1) * 64],
                                                 rhs=XHs[:, c0:c0 + cw], start=True, stop=True)
                            TI = wpA.tile([128, 512], F16, tag="TI")
                            TF = wpA.tile([128, 512], F16, tag="TF")
                            TO = wpA.tile([128, 512], F16, tag="TO")
                            TG = wpA.tile([128, 512], F16, tag="TG")
                            nc.scalar.activation(TI[:, :cw], P[0][:, :cw], AF.Sigmoid, bias=bias('bg0'))
                            nc.scalar.activation(TF[:, :cw], P[1][:, :cw], AF.Sigmoid, bias=bias('bg1'))
                            nc.scalar.activation(TO[:, :cw], P[2][:, :cw], AF.Sigmoid, bias=bias('bg2'))
                            nc.scalar.activation(TG[:, :cw], P[3][:, :cw], AF.Tanh, bias=bias('bg3'))
                            u = wpA.tile([128, 512], F16, tag="u")
                            v = wpA.tile([128, 512], F16, tag="v")
                            nc.vector.scalar_tensor_tensor(u[:, :cw], TF[:, :cw], 0.0,
                                                           Cst[:, c0:c0 + cw], op0=A.bypass, op1=A.mult)
                            nc.vector.scalar_tensor_tensor(v[:, :cw], TI[:, :cw], 0.0,
                                                           TG[:, :cw], op0=A.bypass, op1=A.mult)
                            nc.vector.scalar_tensor_tensor(Cst[:, c0:c0 + cw], u[:, :cw], 0.0,
                                                           v[:, :cw], op0=A.bypass, op1=A.add)
                            tcn = wpA.tile([128, 512], F16, tag="tc")
                            nc.scalar.activation(tcn[:, :cw], Cst[:, c0:c0 + cw], AF.Tanh)
                            nc.vector.scalar_tensor_tensor(XHv[0:64, c0:c0 + cw], TO[0:64, :cw], 0.0,
                                                           tcn[0:64, :cw], op0=A.bypass, op1=A.mult)
                            nc.vector.scalar_tensor_tensor(XHs[64:128, c0:c0 + cw], TO[64:128, :cw], 0.0,
                                                           tcn[64:128, :cw], op0=A.bypass, op1=A.mult)

                # ---- Phase A epilogue (inside stA scope: uses XHv/XHs)
                PT = stA.tile([32, NLP], F16)
                nc.sync.dma_start(PT[:], inputs['pers'][:])
                with (tc.tile_pool(name="psB", bufs=2, space="PSUM") as psB,
                      tc.tile_pool(name="psBs", bufs=2, space="PSUM") as psBs):
                    for (c0, cw) in CHUNKS:
                        pxa = psB.tile([128, 512], F32, tag="pa")
                        nc.tensor.matmul(pxa[:, :cw], lhsT=wsl(wt, 'lin'),
                                         rhs=XHv[0:64, c0:c0 + cw], start=True, stop=True)
                        pxp = psB.tile([128, 512], F32, tag="pb")
                        nc.tensor.matmul(pxp[:, :cw], lhsT=wsl(wt, 'pers'),
                                         rhs=PT[:, c0:c0 + cw], start=True, stop=True)
                        pxs = psB.tile([128, 512], F32, tag="pc")
                        nc.tensor.matmul(pxs[:, :cw], lhsT=wsl(wt, 'lin1'),
                                         rhs=XHs[64:128, c0:c0 + cw], start=True, stop=True)
                        XA = wpA.tile([128, 512], F16, tag="XA")
                        XP = wpA.tile([128, 512], F16, tag="XP")
                        XS = wpA.tile([128, 512], F16, tag="XS")
                        nc.scalar.activation(XA[:, :cw], pxa[:, :cw], AF.Lrelu, bias=bias('b_lin'), alpha=0.01)
                        nc.scalar.activation(XP[:, :cw], pxp[:, :cw], AF.Lrelu, bias=bias('b_pers'), alpha=0.01)
                        nc.scalar.activation(XS[:, :cw], pxs[:, :cw], AF.Lrelu, bias=bias('b_lin1'), alpha=0.01)
                        p0 = psB.tile([128, 512], F32, tag="pa")
                        nc.tensor.matmul(p0[:, :cw], lhsT=wsl(wt, 'lin2a'), rhs=XA[:, :cw], start=True, stop=False)
                        nc.tensor.matmul(p0[:, :cw], lhsT=wsl(wt, 'lin2b'), rhs=XP[:, :cw], start=False, stop=True)
                        p1 = psB.tile([128, 512], F32, tag="pb")
                        nc.tensor.matmul(p1[:, :cw], lhsT=wsl(wt, 'lin3a'), rhs=XS[:, :cw], start=True, stop=False)
                        nc.tensor.matmul(p1[:, :cw], lhsT=wsl(wt, 'lin3b'), rhs=XP[:, :cw], start=False, stop=True)
                        p2 = psB.tile([128, 512], F32, tag="pc")
                        nc.tensor.matmul(p2[:, :cw], lhsT=wsl(wt, 'lin4a'), rhs=XA[:, :cw], start=True, stop=False)
                        nc.tensor.matmul(p2[:, :cw], lhsT=wsl(wt, 'lin4bd'), rhs=XP[:, :cw], start=False, stop=False)
                        nc.tensor.matmul(p2[:, :cw], lhsT=wsl(wt, 'lin4c'), rhs=XS[:, :cw], start=False, stop=True)
                        X0c = wpA.tile([128, 512], F16, tag="X0c")
                        X1c = wpA.tile([128, 512], F16, tag="X1c")
                        X2c = wpA.tile([128, 512], F16, tag="X2c")
                        nc.scalar.activation(X0c[:, :cw], p0[:, :cw], AF.Lrelu, bias=bias('b2'), alpha=0.01)
                        nc.scalar.activation(X1c[:, :cw], p1[:, :cw], AF.Lrelu, bias=bias('b3'), alpha=0.01)
                        nc.scalar.activation(X2c[:, :cw], p2[:, :cw], AF.Lrelu, bias=bias('b4'), alpha=0.01)
                        nc.sync.dma_start(xin[0][:, c0:c0 + cw], X0c[:, :cw])
                        nc.sync.dma_start(xin[1][:, c0:c0 + cw], X1c[:, :cw])
                        nc.sync.dma_start(xin[2][:, c0:c0 + cw], X2c[:, :cw])
                        p6 = psBs.tile([2, 512], F32, tag="p6")
                        nc.tensor.matmul(p6[:, :cw], lhsT=wsl(wt, 'lin6_3'), rhs=X0c[:, :cw],
                                         start=True, stop=False)
                        nc.tensor.matmul(p6[:, :cw], lhsT=wsl(wt, 'lin6_4'), rhs=X1c[:, :cw],
                                         start=False, stop=False)
                        nc.tensor.matmul(p6[:, :cw], lhsT=wsl(wt, 'lin6_5'), rhs=X2c[:, :cw],
                                         start=False, stop=True)
                        o6 = wpA.tile([2, 512], F32, tag="o6")
                        nc.scalar.copy(o6[:, :cw], p6[:, :cw])
                        nc.sync.dma_start(out_parts[3, :, c0:c0 + cw], o6[:, :cw])

            # =============== Phase B ===============
            # global gather-buffer maxima across all stages
            MXLO = MXHI = 1
            for _r in range(R):
                _lw = meta['rel'][_r]['lo_w']; _hw = meta['rel'][_r]['hi_w']
                for _t0 in range(0, NT, GS):
                    _tl = list(range(_t0, min(_t0 + GS, NT)))
                    MXLO = max(MXLO, sum(_lw[t] for t in _tl))
                    MXHI = max(MXHI, sum(_hw[t] for t in _tl))

            def msgprep0(r):
                """hop-0 messages for relation r from xin[r] + AllGather."""
                with (tc.tile_pool(name=f"mp{r}", bufs=1) as mp,
                      tc.tile_pool(name=f"mpw{r}", bufs=2) as mpw,
                      tc.tile_pool(name=f"mpp{r}", bufs=2, space="PSUM") as mpp):
                    XRt = mp.tile([128, NLP], F16)
                    nc.sync.dma_start(XRt[:], xin[r][:])
                    for t in range(NT):
                        tr = mpp.tile([128, 128], F16, tag="tr")
                        nc.tensor.transpose(tr[:], XRt[:, t * 128:(t + 1) * 128],
                                            wsl(wt, 'ident'))
                        m1 = mpw.tile([128, 128], F16, tag="m1")
                        nc.vector.tensor_scalar_mul(m1[:], tr[:], dv(r, t))
                        nc.scalar.dma_start(mld[(r, 0)][t * 128:(t + 1) * 128, :], m1[:])
                nc.gpsimd.collective_compute(
                    "AllGather", A.bypass,
                    replica_groups=[list(range(NCORES))],
                    ins=[mld[(r, 0)][:].opt()], outs=[mfd[(r, 0)][:].opt()],
                )

            def stage(r, h):
                relm = meta['rel'][r]
                lo_w, hi_w = relm['lo_w'], relm['hi_w']
                lo_off, hi_off = [0], [0]
                for t in range(NT):
                    lo_off.append(lo_off[-1] + lo_w[t])
                    hi_off.append(hi_off[-1] + hi_w[t])
                groups = [list(range(t0, min(t0 + GS, NT))) for t0 in range(0, NT, GS)]
                maxlo = max(sum(lo_w[t] for t in tl) for tl in groups)
                maxhi = max(sum(hi_w[t] for t in tl) for tl in groups)
                mf = mfd[(r, h)]

                with (tc.tile_pool(name=f"psT{r}{h}", bufs=2, space="PSUM") as psT,
                      tc.tile_pool(name=f"psG{r}{h}", bufs=2, space="PSUM") as psG):
                    fsrc = fsP.tile([128, NLP], F16, tag="fsrc", bufs=2)
                    fdst = fsP.tile([128, NLP], F16, tag="fdst", bufs=2)
                    if h == 0:
                        XRt = fsP.tile([128, NLP], F16, tag="XRt", bufs=2)
                        nc.sync.dma_start(XRt[:], xin[r][:])
                        for t in range(NT):
                            tr = psT.tile([128, 128], F16, tag="tr")
                            nc.tensor.transpose(tr[:], XRt[:, t * 128:(t + 1) * 128],
                                                wsl(wt, 'ident'))
                            nc.vector.tensor_copy(fsrc[:, t * 128:(t + 1) * 128], tr[:])
                    else:
                        nc.sync.dma_start(fsrc[:], f1d[r][:])

                    iota = wsl(wt, 'iota')
                    for tl in groups:
                        t0 = tl[0]
                        bufs = {}
                        for cls, w_arr, off_arr, mx in (
                                ('lo', lo_w, lo_off, maxlo), ('hi', hi_w, hi_off, maxhi)):
                            nwin = sum(w_arr[t] for t in tl)
                            woff = off_arr[t0]
                            it = gpP.tile([128, (MXLO if cls == 'lo' else MXHI) * 8], I16,
                                          tag=f"idx{cls}", bufs=2)
                            nc.sync.dma_start(it[:, :nwin * 8],
                                              inputs[f'gidx_{cls}_{r}'][:, woff * 8:(woff + nwin) * 8])
                            cv = gpP.tile([128, MXLO if cls == 'lo' else MXHI], F16,
                                          tag=f"cv{cls}", bufs=2)
                            nc.sync.dma_start(cv[:, :nwin],
                                              inputs[f'col_{cls}_{r}'][:, woff:woff + nwin])
                            ib = gpP.tile([128, MXLO if cls == 'lo' else MXHI, 128], F16,
                                          tag=f"ib{cls}", bufs=2)
                            cv_b = cv[:, :nwin].unsqueeze(2).broadcast_to([128, nwin, 128])
                            io_b = iota.unsqueeze(1).broadcast_to([128, nwin, 128])
                            nc.vector.tensor_tensor(ib[:, :nwin, :], cv_b, io_b, A.is_equal)
                            gb = gpP.tile([128, MXLO if cls == 'lo' else MXHI, 128], F16,
                                          tag=f"gb{cls}", bufs=2)
                            in_ap = mf[0:LO_LIM, :] if cls == 'lo' else mf[LO_LIM:NGP, :]
                            for w0 in range(0, nwin, GW):
                                sw = min(GW, nwin - w0)
                                nc.gpsimd.dma_gather(
                                    out_ap=gb[:, w0:w0 + sw, :], in_ap=in_ap,
                                    idxs_ap=it[:, w0 * 8:(w0 + sw) * 8],
                                    num_idxs=sw * 128, num_idxs_reg=sw * 128,
                                    elem_size=H)
                            bufs[cls] = (gb, ib)
                        for t in tl:
                            agg = psG.tile([128, 128], F32, tag="agg")
                            wins = ([('lo', lo_off[t] - lo_off[t0] + w) for w in range(lo_w[t])]
                                    + [('hi', hi_off[t] - hi_off[t0] + w) for w in range(hi_w[t])])
                            for wi, (cls, w) in enumerate(wins):
                                gb, ib = bufs[cls]
                                nc.tensor.matmul(agg[:], lhsT=ib[:, w, :], rhs=gb[:, w, :],
                                                 start=(wi == 0), stop=(wi == len(wins) - 1))
                            nc.vector.scalar_tensor_tensor(
                                fdst[:, t * 128:(t + 1) * 128], agg[:], ndv(r, t),
                                fsrc[:, t * 128:(t + 1) * 128],
                                op0=A.mult, op1=A.add)

                    nc.scalar.dma_start((f1d[r] if h == 0 else f2d[r])[:], fdst[:])
                    if h == 0:
                        # hop-1 messages straight from fdst (f1), then AllGather
                        if True:
                            for t in range(NT):
                                m1 = fsP.tile([128, 128], F16, tag="m1", bufs=2)
                                nc.vector.tensor_scalar_mul(m1[:], fdst[:, t * 128:(t + 1) * 128],
                                                            dv(r, t))
                                nc.scalar.dma_start(mld[(r, 1)][t * 128:(t + 1) * 128, :], m1[:])
                        nc.gpsimd.collective_compute(
                            "AllGather", A.bypass,
                            replica_groups=[list(range(NCORES))],
                            ins=[mld[(r, 1)][:].opt()], outs=[mfd[(r, 1)][:].opt()],
                        )

            def epilogue(r):
                with (tc.tile_pool(name=f"ep{r}", bufs=2) as ep,
                      tc.tile_pool(name=f"psEt{r}", bufs=2, space="PSUM") as psEt,
                      tc.tile_pool(name=f"psEw{r}", bufs=2, space="PSUM") as psEw,
                      tc.tile_pool(name=f"psEs{r}", bufs=1, space="PSUM") as psEs):
                    # psEw: one rotating [128,512] f32 slot (pso/pbj/ph);
                    # psEs: one rotating [4,512] f32 slot (psc/G/po).
                    for (c0, cw) in CHUNKS:
                        nsub = cw // 128
                        F0c = ep.tile([128, 512], F16, tag="F0c")
                        nc.scalar.dma_start(F0c[:, :cw], xin[r][:, c0:c0 + cw])
                        f1c = ep.tile([128, 512], F16, tag="f1c")
                        nc.scalar.dma_start(f1c[:, :cw], f1d[r][:, c0:c0 + cw])
                        f2c = ep.tile([128, 512], F16, tag="f2c")
                        nc.scalar.dma_start(f2c[:, :cw], f2d[r][:, c0:c0 + cw])
                        F1c = ep.tile([128, 512], F16, tag="F1c")
                        F2c = ep.tile([128, 512], F16, tag="F2c")
                        for si in range(nsub):
                            tr = psEt.tile([128, 128], F16, tag="tr")
                            nc.tensor.transpose(tr[:], f1c[:, si * 128:(si + 1) * 128],
                                                wsl(wt, 'ident'))
                            nc.vector.tensor_copy(F1c[:, si * 128:(si + 1) * 128], tr[:])
                            tr2 = psEt.tile([128, 128], F16, tag="tr")
                            nc.tensor.transpose(tr2[:], f2c[:, si * 128:(si + 1) * 128],
                                                wsl(wt, 'ident'))
                            nc.vector.tensor_copy(F2c[:, si * 128:(si + 1) * 128], tr2[:])
                        Bsrc = [F0c[:, :cw], F1c[:, :cw], F2c[:, :cw]]
                        G = psEs.tile([128, 512], F32, tag="G")
                        for o in range(5):
                            pso = psEw.tile([128, 512], F32, tag="big")
                            js = [j for j in range(3) if CTRUE[o][j] != 0.0]
                            for ji, j in enumerate(js):
                                nc.tensor.matmul(pso[:, :cw], lhsT=wsl(wt, f'wf1_{r}_{o}_{j}'),
                                                 rhs=Bsrc[j], start=(ji == 0), stop=(ji == len(js) - 1))
                            To = ep.tile([128, 512], F16, tag="To")
                            nc.scalar.activation(To[:, :cw], pso[:, :cw], AF.Tanh, bias=bias(f'bf1_{r}'))
                            psc = psEs.tile([1, 512], F32, tag="psc", bufs=2)
                            nc.tensor.matmul(psc[:, :cw], lhsT=wsl(wt, f'wf2_{r}'), rhs=To[:, :cw],
                                             start=True, stop=True)
                            eo = ep.tile([1, 512], F16, tag="eo")
                            nc.scalar.activation(eo[:, :cw], psc[:, :cw], AF.Exp)
                            nc.tensor.matmul(G[:, :cw], lhsT=wsl(wt, f'c5_{o}'), rhs=eo[:, :cw],
                                             start=(o == 0), stop=(o == 4))
                        rec = ep.tile([1, 512], F32, tag="rec")
                        nc.vector.reciprocal(rec[:, :cw], G[0:1, :cw])
                        res = ep.tile([128, 512], F16, tag="res")
                        tmp = ep.tile([128, 512], F16, tag="tmp")
                        for j in range(3):
                            gj = ep.tile([1, 512], F16, tag="gj")
                            nc.vector.scalar_tensor_tensor(gj[:, :cw], rec[:, :cw], 0.0,
                                                           G[32 * (j + 1):32 * (j + 1) + 1, :cw],
                                                           op0=A.bypass, op1=A.mult)
                            pbj = psEw.tile([128, 512], F32, tag="big")
                            nc.tensor.matmul(pbj[:, :cw], lhsT=onesf16[:], rhs=gj[:, :cw],
                                             start=True, stop=True)
                            if j == 0:
                                nc.vector.scalar_tensor_tensor(res[:, :cw], Bsrc[j], 0.0, pbj[:, :cw],
                                                               op0=A.bypass, op1=A.mult)
                            else:
                                nc.vector.scalar_tensor_tensor(tmp[:, :cw], Bsrc[j], 0.0, pbj[:, :cw],
                                                               op0=A.bypass, op1=A.mult)
                                nc.vector.scalar_tensor_tensor(res[:, :cw], res[:, :cw], 0.0,
                                                               tmp[:, :cw], op0=A.bypass, op1=A.add)
                        ph = psEw.tile([128, 512], F32, tag="big")
                        nc.tensor.matmul(ph[:, :cw], lhsT=wsl(wt, f'lin5_{r}'), rhs=res[:, :cw],
                                         start=True, stop=True)
                        hall = ep.tile([128, 512], F16, tag="hall")
                        nc.scalar.activation(hall[:, :cw], ph[:, :cw], AF.Lrelu,
                                             bias=bias(f'b5_{r}'), alpha=0.01)
                        po = psEs.tile([2, 512], F32, tag="small")
                        nc.tensor.matmul(po[:, :cw], lhsT=wsl(wt, f'lin6_{r}')[:, 0:2], rhs=hall[:, :cw],
                                         start=True, stop=True)
                        oo = ep.tile([2, 512], F32, tag="oo")
                        nc.scalar.copy(oo[:, :cw], po[:, :cw])
                        nc.scalar.dma_start(out_parts[r, :, c0:c0 + cw], oo[:, :cw])

            with (tc.tile_pool(name="fsP", bufs=1) as fsP,
                  tc.tile_pool(name="gpP", bufs=1) as gpP):
                for r in range(R):
                    msgprep0(r)
                stage(0, 0)
                stage(1, 0)
                stage(2, 0)
                stage(0, 1)
                stage(1, 1)
                stage(2, 1)
                epilogue(0)
                epilogue(1)
                epilogue(2)

    nc.compile()


def kernel(**inp):
    meta, cores = _prep(inp)
    nc = bacc.Bacc("TRN2", target_bir_lowering=False, debug=False, num_devices=NCORES)
    _build(nc, meta)
    res = run_bass_kernel_spmd(nc, [dict(c) for c in cores], core_ids=list(range(NCORES)))
    out = np.zeros((N, C), np.float32)
    b6 = np.asarray(inp['b_lin6'], np.float32)
    for c in range(NCORES):
        parts = res.results[c]["out_parts"]
        out[c * NL:(c + 1) * NL] = parts.sum(axis=0).T[:NL] + b6[None, :]
    return out


if __name__ == "__main__":
    # quick self-run against the reference
    import reference
    inputs = {k: np.asarray(v) for k, v in reference.setup_inputs().items()}
    got = kernel(**inputs)
    exp = np.asarray(reference.reference(**inputs))
    err = np.abs(got - exp).max()
    rel = err / max(np.abs(exp).max(), 1e-9)
    print("abs err:", err, "rel err:", rel)


# revision 13
# speedup vs baseline: 1.0366x; 1.0366x over previous
"""BWGNN_Hetero Trainium2 kernel v2: 8-core SPMD, node-sharded graph parallel.

Structure (per core):
  Phase A: fused fp16 LSTMs (voc+sms packed in one 128-row tile) -> linear
  head -> x_in[0..2] (feat-major fp16) to DRAM + out_parts[3] partials.

  Phase B: 6 (hop, relation) stages interleaved as (0,r0)(0,r1)(0,r2)
  (1,r0)(1,r1) E(r0) (1,r2) E(r1) E(r2) so the gpsimd dma_gather
  descriptor generation (the hard bottleneck, ~8.4ns/edge) streams
  continuously while PE/DVE/ACT work hides underneath. AllGathers for a
  stage are issued as soon as its messages exist (end of the producing
  stage), hiding collective latency.

  Aggregation: fp16 message AllGather -> dma_gather by src (1024-idx
  calls) -> one-hot indicator matmuls (f32 PSUM) as the dst segment sum.
  Indicators are generated on device (DVE is_equal vs an iota row,
  broadcast APs) instead of loaded from HBM.

  Epilogue per relation: single merged pass: transposes -> 5-filter
  scores -> softmax via one [5x4] matmul (den+3 gammas) -> combine ->
  lin5 -> lin6 partials.

Host sums partials, adds b_lin6, unshards.
"""
import sys
sys.path.insert(0, '/opt/trn_rl_repo')
sys.path.insert(0, '/root/problem')

import numpy as np

import concourse.bacc as bacc
import concourse.bass as bass
import concourse.mybir as mybir
import concourse.tile as tile
from concourse.bass_utils import run_bass_kernel_spmd

F32 = mybir.dt.float32
F16 = mybir.dt.float16
I16 = mybir.dt.int16
A = mybir.AluOpType
AF = mybir.ActivationFunctionType

NCORES = 8
N, E, R, T = 50000, 800000, 3, 16
IV, IS, IP, H, C = 64, 64, 32, 128, 2
NL = N // NCORES            # 6250
NT = 49                     # dst tiles per core
NLP = NT * 128              # 6272 padded local nodes
NGP = NLP * NCORES          # 50176 padded global rows in AllGather output
LO_LIM = 32768              # int16 gather index limit
GS = 4                      # dst tiles per gather group
GW = 8                      # windows per dma_gather call (1024 idx ring cap)
CHUNKS = [(i * 512, 512) for i in range(12)] + [(6144, 128)]

CTRUE = [[0.8, -0.5, 0.0],
         [3.0, -3.0, 0.75],
         [0.0, 3.0, -1.5],
         [0.0, 0.0, 0.75],
         [-0.2, 0.5, 0.0]]


def _wrap_idx(idx):
    """[n] int16 -> [128, ceil(n/16)] wrapped (i -> [i%16, i//16]) + replicated x8."""
    n = len(idx)
    L = max(1, (n + 15) // 16)
    a = np.zeros((16, L), np.int16)
    for p in range(16):
        vals = idx[p::16]
        a[p, :len(vals)] = vals
    return np.tile(a, (8, 1))


class WPack:
    def __init__(self):
        self.cols = []
        self.off = 0
        self.slots = {}

    def add(self, name, mat, row0=0):
        mat = np.asarray(mat, np.float32)
        k, m = mat.shape
        assert row0 + k <= 128
        buf = np.zeros((128, m), np.float16)
        buf[row0:row0 + k] = mat.astype(np.float16)
        self.cols.append(buf)
        self.slots[name] = (row0, k, self.off, m)
        self.off += m

    def image(self):
        return np.concatenate(self.cols, axis=1)


def _prep(inp):
    g = {k: np.asarray(v) for k, v in inp.items()}
    wp = WPack()

    groff = {0: 0, 1: 64, 2: 192, 3: 128}   # our gate order [i, f, o, gg] -> torch rows
    lv = np.zeros((128, 256), np.float32)
    ls = np.zeros((128, 256), np.float32)
    for gi in range(4):
        ro = groff[gi]
        lv[0:64, gi * 64:(gi + 1) * 64] = g['Whh_v'][ro:ro + 64, :].T
        lv[64:128, gi * 64:(gi + 1) * 64] = g['Wih_v'][ro:ro + 64, :].T
        ls[0:64, gi * 64:(gi + 1) * 64] = g['Wih_s'][ro:ro + 64, :].T
        ls[64:128, gi * 64:(gi + 1) * 64] = g['Whh_s'][ro:ro + 64, :].T
    wp.add('lstm_v', lv)
    wp.add('lstm_s', ls)
    wp.add('lin', g['W_lin'].T)                       # rows 0:64 (rhs = h_v at base 0)
    wp.add('lin1', g['W_lin1'].T, row0=64)            # rows 64:128 (rhs = h_s at base 64)
    wp.add('pers', g['W_pers'].T)
    wp.add('lin2a', g['W_lin2'][:, 0:128].T)
    wp.add('lin2b', g['W_lin2'][:, 128:256].T)
    wp.add('lin3a', g['W_lin3'][:, 0:128].T)
    wp.add('lin3b', g['W_lin3'][:, 128:256].T)
    wp.add('lin4a', g['W_lin4'][:, 0:128].T)
    wp.add('lin4bd', (g['W_lin4'][:, 128:256] + g['W_lin4'][:, 384:512]).T)
    wp.add('lin4c', g['W_lin4'][:, 256:384].T)
    for r in range(R):
        for o in range(5):
            for j in range(3):
                if CTRUE[o][j] != 0.0:
                    wp.add(f'wf1_{r}_{o}_{j}', (CTRUE[o][j] * g['Wf1'][r]).T)
        wp.add(f'wf2_{r}', g['Wf2'][r][:, None])
        wp.add(f'lin5_{r}', g['W_lin5'][r].T)
    for k in range(6):
        wp.add(f'lin6_{k}', g['W_lin6'][:, k * 128:(k + 1) * 128].T)
    wp.add('ident', np.eye(128, dtype=np.float32))
    # softmax combine rows: per filter o a [1,128] row; den coeff at col 0,
    # gamma_j coeffs at cols 32/64/96 (partition-aligned PSUM rows)
    for o in range(5):
        row = np.zeros((1, 128), np.float32)
        row[0, 0] = 1.0
        for j in range(3):
            row[0, 32 * (j + 1)] = CTRUE[o][j]
        wp.add(f'c5_{o}', row)
    wp.add('iota', np.tile(np.arange(128, dtype=np.float32), (128, 1)))
    wimg = wp.image()

    bcols, blist = {}, []

    def addb(name, vec):
        bcols[name] = len(blist)
        v = np.zeros((128, 1), np.float32)
        v[:len(vec), 0] = np.asarray(vec, np.float32).ravel()
        blist.append(v)

    bv = g['bih_v'] + g['bhh_v']
    bs = g['bih_s'] + g['bhh_s']
    for gi in range(4):
        ro = groff[gi]
        addb(f'bg{gi}', np.concatenate([bv[ro:ro + 64], bs[ro:ro + 64]]))
    addb('b_lin', g['b_lin'])
    addb('b_lin1', g['b_lin1'])
    addb('b_pers', g['b_pers'])
    addb('b2', g['b_lin2'])
    addb('b3', g['b_lin3'])
    addb('b4', g['b_lin4'])
    for r in range(R):
        addb(f'bf1_{r}', g['bf1'][r])
        addb(f'b5_{r}', g['b_lin5'][r])
    bimg = np.concatenate(blist, axis=1)

    src = np.asarray(g['src'], np.int64)
    dst = np.asarray(g['dst'], np.int64)
    gsrc_all = (src // NL) * NLP + (src % NL)

    percore = [dict() for _ in range(NCORES)]
    relmeta = []
    for r in range(R):
        deg = np.bincount(dst[r], minlength=N).astype(np.float32)
        dinv = np.clip(deg, 1.0, None) ** -0.5

        # bucket edges: (core, tile, class)
        per = []
        for c in range(NCORES):
            m = (dst[r] // NL) == c
            sc = gsrc_all[r][m]
            dl = dst[r][m] - c * NL
            tl, col = dl // 128, dl % 128
            tiles = []
            for t in range(NT):
                mt = tl == t
                st_, ct_ = sc[mt], col[mt]
                lo = st_ < LO_LIM
                tiles.append((st_[lo], ct_[lo], st_[~lo] - LO_LIM, ct_[~lo]))
            per.append(tiles)
        # common (max-over-cores) window counts
        lo_w = [max(1, max((len(per[c][t][0]) + 127) // 128 for c in range(NCORES)))
                for t in range(NT)]
        hi_w = [max(1, max((len(per[c][t][2]) + 127) // 128 for c in range(NCORES)))
                for t in range(NT)]
        relmeta.append({'lo_w': lo_w, 'hi_w': hi_w})
        for c in range(NCORES):
            li_s, lc_s, hi_s, hc_s = [], [], [], []
            for t in range(NT):
                li, lc, hi, hc = per[c][t]
                lp = np.zeros(lo_w[t] * 128, np.int64); lp[:len(li)] = li
                lcp = np.full(lo_w[t] * 128, -1, np.int64); lcp[:len(lc)] = lc
                hp = np.zeros(hi_w[t] * 128, np.int64); hp[:len(hi)] = hi
                hcp = np.full(hi_w[t] * 128, -1, np.int64); hcp[:len(hc)] = hc
                li_s.append(lp); lc_s.append(lcp); hi_s.append(hp); hc_s.append(hcp)
            li_s = np.concatenate(li_s); lc_s = np.concatenate(lc_s)
            hi_s = np.concatenate(hi_s); hc_s = np.concatenate(hc_s)

            def mkcol(colarr):
                W = len(colarr) // 128
                return colarr.reshape(W, 128).T.astype(np.float16)
            pc = percore[c]
            pc[f'gidx_lo_{r}'] = _wrap_idx(li_s.astype(np.int16))
            pc[f'gidx_hi_{r}'] = _wrap_idx(hi_s.astype(np.int16))
            pc[f'col_lo_{r}'] = mkcol(lc_s)
            pc[f'col_hi_{r}'] = mkcol(hc_s)
            dp = pc.setdefault('_dinv', np.zeros((128, 2 * R * NT), np.float32))
            dvl = np.ones(NLP, np.float32)
            dvl[:NL] = dinv[c * NL:(c + 1) * NL]
            dp[:, r * NT:(r + 1) * NT] = dvl.reshape(NT, 128).T
            dp[:, R * NT + r * NT:R * NT + (r + 1) * NT] = -dvl.reshape(NT, 128).T

    voc = np.asarray(g['voc_features'], np.float32)
    sms = np.asarray(g['sms_features'], np.float32)
    pers = np.asarray(g['personal_feature'], np.float32)
    cores = []
    for c in range(NCORES):
        pc = percore[c]
        sl = slice(c * NL, (c + 1) * NL)
        vt = np.zeros((T, IV, NLP), np.float16)
        st_ = np.zeros((T, IS, NLP), np.float16)
        vt[:, :, :NL] = voc[sl].transpose(1, 2, 0).astype(np.float16)
        st_[:, :, :NL] = sms[sl].transpose(1, 2, 0).astype(np.float16)
        pt = np.zeros((IP, NLP), np.float16)
        pt[:, :NL] = pers[sl].T.astype(np.float16)
        pc['voc'] = vt
        pc['sms'] = st_
        pc['pers'] = pt
        pc['wpack'] = wimg
        pc['bpack'] = bimg
        pc['dpack'] = pc.pop('_dinv')
        cores.append(pc)
    meta = {
        'wp': wp.slots, 'bcols': bcols, 'rel': relmeta,
        'shapes': {k: v.shape for k, v in cores[0].items()},
        'dtypes': {k: v.dtype for k, v in cores[0].items()},
    }
    return meta, cores


def _build(nc, meta):
    sh, dt = meta['shapes'], meta['dtypes']
    WP, BC = meta['wp'], meta['bcols']
    inputs = {k: nc.dram_tensor(k, list(sh[k]), mybir.dt.from_np(np.dtype(dt[k])),
                                kind="ExternalInput") for k in sh}
    out_parts = nc.dram_tensor("out_parts", [4, 2, NLP], F32, kind="ExternalOutput")

    xin = [nc.dram_tensor(f"xin{r}", [128, NLP], F16, kind="Internal")
           for r in range(R)]
    f1d = [nc.dram_tensor(f"f1d{r}", [128, NLP], F16, kind="Internal")
           for r in range(R)]
    f2d = [nc.dram_tensor(f"f2d{r}", [128, NLP], F16, kind="Internal")
           for r in range(R)]
    mld = {(r, h): nc.dram_tensor(f"ml{r}{h}", [NLP, H], F16, kind="Internal")
           for r in range(R) for h in range(2)}
    mfd = {(r, h): nc.dram_tensor(f"mf{r}{h}", [NGP, H], F16, kind="Internal",
                                  addr_space="Shared")
           for r in range(R) for h in range(2)}

    def wsl(wt, name):
        r0, k, off, m = WP[name]
        return wt[r0:r0 + k, off:off + m]

    with tile.TileContext(nc) as tc:
        with (
            tc.tile_pool(name="const", bufs=1) as cpool,
        ):
            wt = cpool.tile([128, sh['wpack'][1]], F16)
            nc.sync.dma_start(wt[:], inputs['wpack'][:])
            bt = cpool.tile([128, sh['bpack'][1]], F32)
            nc.sync.dma_start(bt[:], inputs['bpack'][:])
            dpt = cpool.tile([128, 2 * R * NT], F32)
            nc.sync.dma_start(dpt[:], inputs['dpack'][:])
            onesf16 = cpool.tile([1, 128], F16)
            nc.vector.memset(onesf16[:], 1.0)

            def bias(name):
                return bt[:, BC[name]:BC[name] + 1]

            def dv(r, t):
                return dpt[:, r * NT + t:r * NT + t + 1]

            def ndv(r, t):
                return dpt[:, R * NT + r * NT + t:R * NT + r * NT + t + 1]

            # =============== Phase A ===============
            with (tc.tile_pool(name="stA", bufs=1) as stA,
                  tc.tile_pool(name="wpA", bufs=2) as wpA):
                XHv = stA.tile([128, NLP], F16)
                XHs = stA.tile([128, NLP], F16)
                Cst = stA.tile([128, NLP], F16)
                nc.vector.memset(XHv[0:64, :], 0.0)
                nc.vector.memset(XHs[64:128, :], 0.0)
                nc.vector.memset(Cst[:], 0.0)
                with tc.tile_pool(name="psA", bufs=2, space="PSUM") as psA:
                    for t in range(T):
                        nc.sync.dma_start(XHv[64:128, :], inputs['voc'][t])
                        nc.sync.dma_start(XHs[0:64, :], inputs['sms'][t])
                        for (c0, cw) in CHUNKS:
                            P = [psA.tile([128, 512], F32, tag=f"g{gi}", name=f"Pg{gi}") for gi in range(4)]
                            for gi in range(4):
                                nc.tensor.matmul(P[gi][0:64, :cw],
                                                 lhsT=wsl(wt, 'lstm_v')[:, gi * 64:(gi + 1) * 64],
                                                 rhs=XHv[:, c0:c0 + cw], start=True, stop=True)
                                nc.tensor.matmul(P[gi][64:128, :cw],
                                                 lhsT=wsl(wt, 'lstm_s')[:, gi * 64:(gi + 1) * 64],
                                                 rhs=XHs[:, c0:c0 + cw], start=True, stop=True)
                            TI = wpA.tile([128, 512], F16, tag="TI")
                            TF = wpA.tile([128, 512], F16, tag="TF")
                            TO = wpA.tile([128, 512], F16, tag="TO")
                            TG = wpA.tile([128, 512], F16, tag="TG")
                            nc.scalar.activation(TI[:, :cw], P[0][:, :cw], AF.Sigmoid, bias=bias('bg0'))
                            nc.scalar.activation(TF[:, :cw], P[1][:, :cw], AF.Sigmoid, bias=bias('bg1'))
                            nc.scalar.activation(TO[:, :cw], P[2][:, :cw], AF.Sigmoid, bias=bias('bg2'))
                            nc.scalar.activation(TG[:, :cw], P[3][:, :cw], AF.Tanh, bias=bias('bg3'))
                            u = wpA.tile([128, 512], F16, tag="u")
                            v = wpA.tile([128, 512], F16, tag="v")
                            nc.vector.scalar_tensor_tensor(u[:, :cw], TF[:, :cw], 0.0,
                                                           Cst[:, c0:c0 + cw], op0=A.bypass, op1=A.mult)
                            nc.vector.scalar_tensor_tensor(v[:, :cw], TI[:, :cw], 0.0,
                                                           TG[:, :cw], op0=A.bypass, op1=A.mult)
                            nc.vector.scalar_tensor_tensor(Cst[:, c0:c0 + cw], u[:, :cw], 0.0,
                                                           v[:, :cw], op0=A.bypass, op1=A.add)
                            tcn = wpA.tile([128, 512], F16, tag="tc")
                            nc.scalar.activation(tcn[:, :cw], Cst[:, c0:c0 + cw], AF.Tanh)
                            nc.vector.scalar_tensor_tensor(XHv[0:64, c0:c0 + cw], TO[0:64, :cw], 0.0,
                                                           tcn[0:64, :cw], op0=A.bypass, op1=A.mult)
                            nc.vector.scalar_tensor_tensor(XHs[64:128, c0:c0 + cw], TO[64:128, :cw], 0.0,
                                                           tcn[64:128, :cw], op0=A.bypass, op1=A.mult)

                # ---- Phase A epilogue (inside stA scope: uses XHv/XHs)
                PT = stA.tile([32, NLP], F16)
                nc.sync.dma_start(PT[:], inputs['pers'][:])
                with (tc.tile_pool(name="psB", bufs=2, space="PSUM") as psB,
                      tc.tile_pool(name="psBs", bufs=2, space="PSUM") as psBs):
                    for (c0, cw) in CHUNKS:
                        pxa = psB.tile([128, 512], F32, tag="pa")
                        nc.tensor.matmul(pxa[:, :cw], lhsT=wsl(wt, 'lin'),
                                         rhs=XHv[0:64, c0:c0 + cw], start=True, stop=True)
                        pxp = psB.tile([128, 512], F32, tag="pb")
                        nc.tensor.matmul(pxp[:, :cw], lhsT=wsl(wt, 'pers'),
                                         rhs=PT[:, c0:c0 + cw], start=True, stop=True)
                        pxs = psB.tile([128, 512], F32, tag="pc")
                        nc.tensor.matmul(pxs[:, :cw], lhsT=wsl(wt, 'lin1'),
                                         rhs=XHs[64:128, c0:c0 + cw], start=True, stop=True)
                        XA = wpA.tile([128, 512], F16, tag="XA")
                        XP = wpA.tile([128, 512], F16, tag="XP")
                        XS = wpA.tile([128, 512], F16, tag="XS")
                        nc.scalar.activation(XA[:, :cw], pxa[:, :cw], AF.Lrelu, bias=bias('b_lin'), alpha=0.01)
                        nc.scalar.activation(XP[:, :cw], pxp[:, :cw], AF.Lrelu, bias=bias('b_pers'), alpha=0.01)
                        nc.scalar.activation(XS[:, :cw], pxs[:, :cw], AF.Lrelu, bias=bias('b_lin1'), alpha=0.01)
                        p0 = psB.tile([128, 512], F32, tag="pa")
                        nc.tensor.matmul(p0[:, :cw], lhsT=wsl(wt, 'lin2a'), rhs=XA[:, :cw], start=True, stop=False)
                        nc.tensor.matmul(p0[:, :cw], lhsT=wsl(wt, 'lin2b'), rhs=XP[:, :cw], start=False, stop=True)
                        p1 = psB.tile([128, 512], F32, tag="pb")
                        nc.tensor.matmul(p1[:, :cw], lhsT=wsl(wt, 'lin3a'), rhs=XS[:, :cw], start=True, stop=False)
                        nc.tensor.matmul(p1[:, :cw], lhsT=wsl(wt, 'lin3b'), rhs=XP[:, :cw], start=False, stop=True)
                        p2 = psB.tile([128, 512], F32, tag="pc")
                        nc.tensor.matmul(p2[:, :cw], lhsT=wsl(wt, 'lin4a'), rhs=XA[:, :cw], start=True, stop=False)
                        nc.tensor.matmul(p2[:, :cw], lhsT=wsl(wt, 'lin4bd'), rhs=XP[:, :cw], start=False, stop=False)
                        nc.tensor.matmul(p2[:, :cw], lhsT=wsl(wt, 'lin4c'), rhs=XS[:, :cw], start=False, stop=True)
                        X0c = wpA.tile([128, 512], F16, tag="X0c")
                        X1c = wpA.tile([128, 512], F16, tag="X1c")
                        X2c = wpA.tile([128, 512], F16, tag="X2c")
                        nc.scalar.activation(X0c[:, :cw], p0[:, :cw], AF.Lrelu, bias=bias('b2'), alpha=0.01)
                        nc.scalar.activation(X1c[:, :cw], p1[:, :cw], AF.Lrelu, bias=bias('b3'), alpha=0.01)
                        nc.scalar.activation(X2c[:, :cw], p2[:, :cw], AF.Lrelu, bias=bias('b4'), alpha=0.01)
                        nc.sync.dma_start(xin[0][:, c0:c0 + cw], X0c[:, :cw])
                        nc.sync.dma_start(xin[1][:, c0:c0 + cw], X1c[:, :cw])
                        nc.sync.dma_start(xin[2][:, c0:c0 + cw], X2c[:, :cw])
                        p6 = psBs.tile([2, 512], F32, tag="p6")
                        nc.tensor.matmul(p6[:, :cw], lhsT=wsl(wt, 'lin6_3'), rhs=X0c[:, :cw],
                                         start=True, stop=False)
                        nc.tensor.matmul(p6[:, :cw], lhsT=wsl(wt, 'lin6_4'), rhs=X1c[:, :cw],
                                         start=False, stop=False)
                        nc.tensor.matmul(p6[:, :cw], lhsT=wsl(wt, 'lin6_5'), rhs=X2c[:, :cw],
                                         start=False, stop=True)
                        o6 = wpA.tile([2, 512], F32, tag="o6")
                        nc.scalar.copy(o6[:, :cw], p6[:, :cw])
                        nc.sync.dma_start(out_parts[3, :, c0:c0 + cw], o6[:, :cw])

            # =============== Phase B ===============
            # global gather-buffer maxima across all stages
            MXLO = MXHI = 1
            for _r in range(R):
                _lw = meta['rel'][_r]['lo_w']; _hw = meta['rel'][_r]['hi_w']
                for _t0 in range(0, NT, GS):
                    _tl = list(range(_t0, min(_t0 + GS, NT)))
                    MXLO = max(MXLO, sum(_lw[t] for t in _tl))
                    MXHI = max(MXHI, sum(_hw[t] for t in _tl))

            def msgprep0(r):
                """hop-0 messages for relation r from xin[r] + AllGather."""
                XRt = fsP.tile([128, NLP], F16, tag="XRt", bufs=2)
                nc.sync.dma_start(XRt[:], xin[r][:])
                for t in range(NT):
                    tr = psTP.tile([128, 128], F16, tag="tr", bufs=2)
                    nc.tensor.transpose(tr[:], XRt[:, t * 128:(t + 1) * 128],
                                        wsl(wt, 'ident'))
                    m1 = fsP.tile([128, 128], F16, tag="m1", bufs=2)
                    nc.vector.tensor_scalar_mul(m1[:], tr[:], dv(r, t))
                    nc.scalar.dma_start(mld[(r, 0)][t * 128:(t + 1) * 128, :], m1[:])
                nc.gpsimd.collective_compute(
                    "AllGather", A.bypass,
                    replica_groups=[list(range(NCORES))],
                    ins=[mld[(r, 0)][:].opt()], outs=[mfd[(r, 0)][:].opt()],
                )

            def stage(r, h, epi=None):
                relm = meta['rel'][r]
                lo_w, hi_w = relm['lo_w'], relm['hi_w']
                lo_off, hi_off = [0], [0]
                for t in range(NT):
                    lo_off.append(lo_off[-1] + lo_w[t])
                    hi_off.append(hi_off[-1] + hi_w[t])
                groups = [list(range(t0, min(t0 + GS, NT))) for t0 in range(0, NT, GS)]
                maxlo = max(sum(lo_w[t] for t in tl) for tl in groups)
                maxhi = max(sum(hi_w[t] for t in tl) for tl in groups)
                mf = mfd[(r, h)]

                if True:
                    psT, psG = psTP, psGP
                    fsrc = fsP.tile([128, NLP], F16, tag="fsrc", bufs=2)
                    fdst = fsP.tile([128, NLP], F16, tag="fdst", bufs=2)
                    if h == 0:
                        XRt = fsP.tile([128, NLP], F16, tag="XRt", bufs=2)
                        nc.sync.dma_start(XRt[:], xin[r][:])
                        for t in range(NT):
                            tr = psT.tile([128, 128], F16, tag="tr", bufs=2)
                            nc.tensor.transpose(tr[:], XRt[:, t * 128:(t + 1) * 128],
                                                wsl(wt, 'ident'))
                            nc.vector.tensor_copy(fsrc[:, t * 128:(t + 1) * 128], tr[:])
                    else:
                        nc.sync.dma_start(fsrc[:], f1d[r][:])

                    iota = wsl(wt, 'iota')
                    for tl in groups:
                        t0 = tl[0]
                        bufs = {}
                        for cls, w_arr, off_arr, mx in (
                                ('lo', lo_w, lo_off, maxlo), ('hi', hi_w, hi_off, maxhi)):
                            nwin = sum(w_arr[t] for t in tl)
                            woff = off_arr[t0]
                            it = gpP.tile([128, (MXLO if cls == 'lo' else MXHI) * 8], I16,
                                          tag=f"idx{cls}", bufs=2)
                            nc.sync.dma_start(it[:, :nwin * 8],
                                              inputs[f'gidx_{cls}_{r}'][:, woff * 8:(woff + nwin) * 8])
                            cv = gpP.tile([128, MXLO if cls == 'lo' else MXHI], F16,
                                          tag=f"cv{cls}", bufs=2)
                            nc.sync.dma_start(cv[:, :nwin],
                                              inputs[f'col_{cls}_{r}'][:, woff:woff + nwin])
                            ib = gpP.tile([128, MXLO if cls == 'lo' else MXHI, 128], F16,
                                          tag=f"ib{cls}", bufs=2)
                            cv_b = cv[:, :nwin].unsqueeze(2).broadcast_to([128, nwin, 128])
                            io_b = iota.unsqueeze(1).broadcast_to([128, nwin, 128])
                            nc.vector.tensor_tensor(ib[:, :nwin, :], cv_b, io_b, A.is_equal)
                            gb = gpP.tile([128, MXLO if cls == 'lo' else MXHI, 128], F16,
                                          tag=f"gb{cls}", bufs=2)
                            in_ap = mf[0:LO_LIM, :] if cls == 'lo' else mf[LO_LIM:NGP, :]
                            for w0 in range(0, nwin, GW):
                                sw = min(GW, nwin - w0)
                                nc.gpsimd.dma_gather(
                                    out_ap=gb[:, w0:w0 + sw, :], in_ap=in_ap,
                                    idxs_ap=it[:, w0 * 8:(w0 + sw) * 8],
                                    num_idxs=sw * 128, num_idxs_reg=sw * 128,
                                    elem_size=H)
                            bufs[cls] = (gb, ib)
                        for t in tl:
                            agg = psG.tile([128, 128], F32, tag="agg", bufs=2)
                            wins = ([('lo', lo_off[t] - lo_off[t0] + w) for w in range(lo_w[t])]
                                    + [('hi', hi_off[t] - hi_off[t0] + w) for w in range(hi_w[t])])
                            for wi, (cls, w) in enumerate(wins):
                                gb, ib = bufs[cls]
                                nc.tensor.matmul(agg[:], lhsT=ib[:, w, :], rhs=gb[:, w, :],
                                                 start=(wi == 0), stop=(wi == len(wins) - 1))
                            nc.vector.scalar_tensor_tensor(
                                fdst[:, t * 128:(t + 1) * 128], agg[:], ndv(r, t),
                                fsrc[:, t * 128:(t + 1) * 128],
                                op0=A.mult, op1=A.add)
                        if epi is not None:
                            gi = groups.index(tl)
                            if gi < len(epi):
                                epi[gi]()

                    if epi is not None:
                        for fn in epi[len(groups):]:
                            fn()
                    nc.scalar.dma_start((f1d[r] if h == 0 else f2d[r])[:], fdst[:])
                    if h == 0:
                        # hop-1 messages straight from fdst (f1), then AllGather
                        if True:
                            for t in range(NT):
                                m1 = fsP.tile([128, 128], F16, tag="m1", bufs=2)
                                nc.vector.tensor_scalar_mul(m1[:], fdst[:, t * 128:(t + 1) * 128],
                                                            dv(r, t))
                                nc.scalar.dma_start(mld[(r, 1)][t * 128:(t + 1) * 128, :], m1[:])
                        nc.gpsimd.collective_compute(
                            "AllGather", A.bypass,
                            replica_groups=[list(range(NCORES))],
                            ins=[mld[(r, 1)][:].opt()], outs=[mfd[(r, 1)][:].opt()],
                        )

            def epilogue(r):
                chunks = []
                ep, psEt, psEw, psEs = epP, psTP, psEwP, psEsP

                def mkchunk(c0, cw):
                    def emit():
                        nsub = cw // 128
                        F0c = ep.tile([128, 512], F16, tag="F0c", bufs=2)
                        nc.scalar.dma_start(F0c[:, :cw], xin[r][:, c0:c0 + cw])
                        f1c = ep.tile([128, 512], F16, tag="f1c", bufs=2)
                        nc.scalar.dma_start(f1c[:, :cw], f1d[r][:, c0:c0 + cw])
                        f2c = ep.tile([128, 512], F16, tag="f2c", bufs=2)
                        nc.scalar.dma_start(f2c[:, :cw], f2d[r][:, c0:c0 + cw])
                        F1c = ep.tile([128, 512], F16, tag="F1c", bufs=2)
                        F2c = ep.tile([128, 512], F16, tag="F2c", bufs=2)
                        for si in range(nsub):
                            tr = psEt.tile([128, 128], F16, tag="tr", bufs=2)
                            nc.tensor.transpose(tr[:], f1c[:, si * 128:(si + 1) * 128],
                                                wsl(wt, 'ident'))
                            nc.vector.tensor_copy(F1c[:, si * 128:(si + 1) * 128], tr[:])
                            tr2 = psEt.tile([128, 128], F16, tag="tr", bufs=2)
                            nc.tensor.transpose(tr2[:], f2c[:, si * 128:(si + 1) * 128],
                                                wsl(wt, 'ident'))
                            nc.vector.tensor_copy(F2c[:, si * 128:(si + 1) * 128], tr2[:])
                        Bsrc = [F0c[:, :cw], F1c[:, :cw], F2c[:, :cw]]
                        G = psEs.tile([128, 512], F32, tag="G")
                        for o in range(5):
                            pso = psEw.tile([128, 512], F32, tag="big", bufs=2)
                            js = [j for j in range(3) if CTRUE[o][j] != 0.0]
                            for ji, j in enumerate(js):
                                nc.tensor.matmul(pso[:, :cw], lhsT=wsl(wt, f'wf1_{r}_{o}_{j}'),
                                                 rhs=Bsrc[j], start=(ji == 0), stop=(ji == len(js) - 1))
                            To = ep.tile([128, 512], F16, tag="To", bufs=2)
                            nc.scalar.activation(To[:, :cw], pso[:, :cw], AF.Tanh, bias=bias(f'bf1_{r}'))
                            psc = psEs.tile([2, 512], F32, tag="small")
                            nc.tensor.matmul(psc[0:1, :cw], lhsT=wsl(wt, f'wf2_{r}'), rhs=To[:, :cw],
                                             start=True, stop=True)
                            eo = ep.tile([1, 512], F16, tag="eo", bufs=2)
                            nc.scalar.activation(eo[:, :cw], psc[0:1, :cw], AF.Exp)
                            nc.tensor.matmul(G[:, :cw], lhsT=wsl(wt, f'c5_{o}'), rhs=eo[:, :cw],
                                             start=(o == 0), stop=(o == 4))
                        rec = ep.tile([1, 512], F32, tag="rec", bufs=2)
                        nc.vector.reciprocal(rec[:, :cw], G[0:1, :cw])
                        res = ep.tile([128, 512], F16, tag="res", bufs=2)
                        tmp = ep.tile([128, 512], F16, tag="tmp", bufs=2)
                        for j in range(3):
                            gj = ep.tile([1, 512], F16, tag="gj", bufs=2)
                            nc.vector.scalar_tensor_tensor(gj[:, :cw], rec[:, :cw], 0.0,
                                                           G[32 * (j + 1):32 * (j + 1) + 1, :cw],
                                                           op0=A.bypass, op1=A.mult)
                            pbj = psEw.tile([128, 512], F32, tag="big", bufs=2)
                            nc.tensor.matmul(pbj[:, :cw], lhsT=onesf16[:], rhs=gj[:, :cw],
                                             start=True, stop=True)
                            if j == 0:
                                nc.vector.scalar_tensor_tensor(res[:, :cw], Bsrc[j], 0.0, pbj[:, :cw],
                                                               op0=A.bypass, op1=A.mult)
                            else:
                                nc.vector.scalar_tensor_tensor(tmp[:, :cw], Bsrc[j], 0.0, pbj[:, :cw],
                                                               op0=A.bypass, op1=A.mult)
                                nc.vector.scalar_tensor_tensor(res[:, :cw], res[:, :cw], 0.0,
                                                               tmp[:, :cw], op0=A.bypass, op1=A.add)
                        ph = psEw.tile([128, 512], F32, tag="big", bufs=2)
                        nc.tensor.matmul(ph[:, :cw], lhsT=wsl(wt, f'lin5_{r}'), rhs=res[:, :cw],
                                         start=True, stop=True)
                        hall = ep.tile([128, 512], F16, tag="hall", bufs=2)
                        nc.scalar.activation(hall[:, :cw], ph[:, :cw], AF.Lrelu,
                                             bias=bias(f'b5_{r}'), alpha=0.01)
                        po = psEs.tile([2, 512], F32, tag="small")
                        nc.tensor.matmul(po[0:2, :cw], lhsT=wsl(wt, f'lin6_{r}')[:, 0:2], rhs=hall[:, :cw],
                                         start=True, stop=True)
                        oo = ep.tile([2, 512], F32, tag="oo", bufs=2)
                        nc.scalar.copy(oo[:, :cw], po[0:2, :cw])
                        nc.scalar.dma_start(out_parts[r, :, c0:c0 + cw], oo[:, :cw])
                    return emit

                for (c0, cw) in CHUNKS:
                    chunks.append(mkchunk(c0, cw))
                return chunks

            with (tc.tile_pool(name="fsP", bufs=1) as fsP,
                  tc.tile_pool(name="gpP", bufs=1) as gpP,
                  tc.tile_pool(name="epP", bufs=1) as epP,
                  tc.tile_pool(name="psTP", bufs=1, space="PSUM") as psTP,
                  tc.tile_pool(name="psGP", bufs=1, space="PSUM") as psGP,
                  tc.tile_pool(name="psEwP", bufs=1, space="PSUM") as psEwP,
                  tc.tile_pool(name="psEsP", bufs=1, space="PSUM") as psEsP):
                for r in range(R):
                    msgprep0(r)
                stage(0, 0)
                stage(1, 0)
                stage(2, 0)
                stage(0, 1)
                stage(1, 1, epi=epilogue(0))
                stage(2, 1, epi=epilogue(1))
                for fn in epilogue(2):
                    fn()

    nc.compile()


def kernel(**inp):
    meta, cores = _prep(inp)
    nc = bacc.Bacc("TRN2", target_bir_lowering=False, debug=False, num_devices=NCORES)
    _build(nc, meta)
    res = run_bass_kernel_spmd(nc, [dict(c) for c in cores], core_ids=list(range(NCORES)))
    out = np.zeros((N, C), np.float32)
    b6 = np.asarray(inp['b_lin6'], np.float32)
    for c in range(NCORES):
        parts = res.results[c]["out_parts"]
        out[c * NL:(c + 1) * NL] = parts.sum(axis=0).T[:NL] + b6[None, :]
    return out


if __name__ == "__main__":
    # quick self-run against the reference
    import reference
    inputs = {k: np.asarray(v) for k, v in reference.setup_inputs().items()}
    got = kernel(**inputs)
    exp = np.asarray(reference.reference(**inputs))
    err = np.abs(got - exp).max()
    rel = err / max(np.abs(exp).max(), 1e-9)
    print("abs err:", err, "rel err:", rel)
